# revision 53
# baseline (speedup 1.0000x reference)
"""GCN (3-layer GCNConv + mean-pool + MLP head) Trainium2 Bass kernel, 8 NeuronCores.

Strategy (graph/data parallel, per sharding hint):
  - Destination nodes are partitioned into 8 contiguous blocks (one per core);
    each core owns SHARD=12544 destinations = 98 windows of 128.
  - Node features live in DRAM tables with 256B-stride rows ([NPAD, 128]
    bf16, first F_in columns real) so the custom dma_gather instruction
    (InstDMAGatherAnt) can fetch thousands of source rows per instruction.
    Gathers fetch only F_in*2 bytes per row (small-elem; the bass-level 256B
    elem assert is a transpose-mode restriction and is relaxed for the
    non-transpose path — only the row STRIDE must be a 256B multiple).
    Chunks of <=5120 idxs are spread over the 4 SWDGE queues by greedy
    idx-count balance so the Q7 descriptor generation (2 cores per queue)
    runs 4-wide; per-bucket edges are sorted by source row for HBM locality.
  - Symmetric GCN norm factoring: norm(e) = dis[src]*dis[dst] with
    dis = deg^-1/2. The layer tables store h~ = dis*h (the src factor); the
    dst factor is applied per window: an optional bias matmul uses
    lhsT = sqrt(deg_d) rows, and the storage activation uses scale = 1/deg
    (dis*relu(dis*p2) = relu(dis^2*p2)); the last layer uses scale = dis.
    This removes the per-edge norm multiply entirely.
  - Self-loops are NOT gathered: each window's self contribution is one
    sequential 32KB row DMA (xself input / the previous layer's own shard
    buffer) + an identity matmul that also opens the PSUM accumulation.
  - The node id space is remapped segment-major into 4 regions of <=28672
    rows so gather indices fit int16; edges are bucketed DENSELY by (piece
    of 7 dst windows, source segment) with window sub-ranges padded only to
    the max count over cores (SPMD-uniform), not to 128-column boundaries.
    A column may span two windows: the later window's edges store
    edl = dloc + 128 and are matched against iota 128..255 in a per-column
    'head' S build; each bucket gets ONE merged is_equal S build.
  - Per window a one-hot S matmul chain accumulates messages in PSUM
    (aggregate-then-transform), then ReLU with per-partition scale.
  - Layer outputs are written as FULL 256B rows into a compact shard buffer
    and AllGathered per segment directly into the next layer's padded gather
    table (no separate expand step). The next layer's first piece is
    pregathered (segs 0-2) before the last AllGather trigger so the SWDGE
    queues stay busy across layer boundaries.
  - Layer-3 output is mean-pooled per graph via one-hot matmuls into a PSUM
    accumulator, AllReduced, and the tiny FC head runs replicated.
"""

import os
import sys
from dataclasses import dataclass

import numpy as np
import ml_dtypes

for _p in ("/opt/trn_rl_repo", "/root/.axon_site/_ro/trn_rl_repo"):
    if os.path.isdir(_p) and _p not in sys.path:
        sys.path.insert(0, _p)

bf16 = ml_dtypes.bfloat16
P = 128
N = 100000
G = 128
F = (40, 40, 80, 160)
HID = 128
NCORES = 8
SHARD = 12544
NW = 98                       # windows per core
CW = 7                        # windows per piece
NP = NW // CW                 # 14 pieces
SEG_W = (28, 28, 28, 14)      # windows per segment
SEG_NODES = tuple(w * P for w in SEG_W)            # 3584,3584,3584,1792
SEG_WSTART = (0, 28, 56, 84)
SEG_START = tuple(w * P for w in SEG_WSTART)       # node offset within shard
REG_SIZE = tuple(NCORES * n for n in SEG_NODES)    # 28672*3, 14336
REG_BASE = (0, 28672, 57344, 86016)
NPAD = NCORES * SHARD         # 100352
NSEG = 4


# ---------------------------------------------------------------- host prep

def _remap_rows():
    """node id -> segment-major global table row."""
    v = np.arange(NPAD, dtype=np.int64)
    c, r = v // SHARD, v % SHARD
    s = np.minimum(r // SEG_NODES[0], 3)
    row = (np.asarray(REG_BASE)[s] + c * np.asarray(SEG_NODES)[s]
           + (r - np.asarray(SEG_START)[s]))
    return row


@dataclass
class Structure:
    ecnt: np.ndarray      # [NW, NSEG] max real-edge count per (window, seg)
    totcol: int
    totslot: int
    cmax: int             # max columns in a piece
    sbmax: int            # max bucket columns (S tile width is sbmax+CW-1)
    piece_col0: list      # per piece: first global column
    piece_ncol: list      # per piece: total columns
    gath: list            # per piece: list over seg of (local col off, ncols)
    wruns: list           # per (piece, wi): (msgs col, ncols, seg, S col)
    heads: list           # per piece: per seg: bucket-local head columns
    bucket_col0: np.ndarray   # [NP, NSEG] global first column of bucket
    colw: np.ndarray      # [totcol] window-in-piece owning the col's 1st slot
    has_bias: bool = True

    def key(self):
        return (self.ecnt.tobytes(), self.totcol, self.cmax, self.has_bias)


def build_structure(ecnt):
    """Static (SPMD-uniform) dense layout: buckets are (piece, seg); window
    sub-ranges inside a bucket are NOT padded to column multiples (only the
    whole bucket is). A column may span two windows; the later window's
    edges carry edl = dloc + 128 and are matched against iota 128..255 in a
    per-column 'head' S build appended after the bucket's S columns."""
    piece_col0, piece_ncol, gath, wruns, heads = [], [], [], [], []
    bucket_col0 = np.zeros((NP, NSEG), np.int64)
    colw_all = []
    col = 0
    for p in range(NP):
        piece_col0.append(col)
        pg = []
        local = 0
        pruns = [[] for _ in range(CW)]
        pheads = []
        for s in range(NSEG):
            off = np.concatenate(
                [[0], np.cumsum(ecnt[p * CW:(p + 1) * CW, s])]).astype(np.int64)
            btot = int(off[-1])
            bcols = (btot + P - 1) // P
            bucket_col0[p, s] = col + local
            pg.append((local, bcols))
            cw0 = np.searchsorted(off, np.arange(bcols) * P,
                                  side="right") - 1
            colw_all.append(cw0)
            sheads = []
            for wi in range(CW):
                o0, o1 = int(off[wi]), int(off[wi + 1])
                if o1 == o0:
                    continue
                c0, r0 = divmod(o0, P)
                c1 = (o1 - 1) // P
                if r0:
                    d = wi - int(cw0[c0])
                    assert 0 <= d <= 1, (p, s, wi, d)
                    if d == 1:
                        # spanning column: S col appended after the bucket's
                        sheads.append(c0)
                        pruns[wi].append(
                            (local + c0, 1, s, bcols + len(sheads) - 1))
                    else:
                        # window starts mid-column but owns the column start
                        pruns[wi].append((local + c0, 1, s, c0))
                ms = c0 + (1 if r0 else 0)
                if c1 >= ms:
                    pruns[wi].append((local + ms, c1 - ms + 1, s, ms))
            pheads.append(sheads)
            local += bcols
        piece_ncol.append(local)
        gath.append(pg)
        wruns.extend(pruns)
        heads.append(pheads)
        col += local
    totcol = col
    colw = np.concatenate(colw_all) if colw_all else np.zeros(0, np.int64)
    assert len(colw) == totcol
    sbmax = max(g[1] for pg in gath for g in pg)
    return Structure(ecnt=ecnt, totcol=totcol, totslot=totcol * P,
                     cmax=max(piece_ncol), sbmax=sbmax, piece_col0=piece_col0,
                     piece_ncol=piece_ncol, gath=gath, wruns=wruns,
                     heads=heads, bucket_col0=bucket_col0, colw=colw)


def build_host_data(inp):
    src = np.asarray(inp["edge_index"][0]).astype(np.int64).ravel()
    dst = np.asarray(inp["edge_index"][1]).astype(np.int64).ravel()
    batch = np.asarray(inp["batch"]).astype(np.int64).ravel()
    deg = (np.bincount(dst, minlength=N) + 1).astype(np.float64)
    dis = 1.0 / np.sqrt(deg)
    # self-loops are NOT gathered: each window's self contribution is added
    # via a sequential self-row DMA + identity matmul instead.
    srcA = src
    dstA = dst

    remap = _remap_rows()
    srow = remap[srcA]
    seg = np.minimum(srow // REG_SIZE[0], 3)
    lidx = srow - np.asarray(REG_BASE)[seg]            # int16-safe (<28672)

    core = dstA // SHARD
    r = dstA % SHARD
    w = r // P
    dloc = r % P
    p = w // CW

    # per-core per-(w,seg) counts -> uniform window sub-ranges (max of cores)
    kid_full = ((core * NW + w) * NSEG + seg)
    cnt = np.bincount(kid_full, minlength=NCORES * NW * NSEG).reshape(
        NCORES, NW, NSEG)
    ecnt = cnt.max(axis=0).astype(np.int64)            # [NW, NSEG]
    st = build_structure(ecnt)
    st.has_bias = any(
        np.any(np.asarray(inp[k]) != 0) for k in ("b1", "b2", "b3"))

    # per-(w,s): global first slot of the window's sub-range
    wsbase = np.zeros((NW, NSEG), np.int64)
    for pp in range(NP):
        for s in range(NSEG):
            o = 0
            for wi in range(CW):
                ww = pp * CW + wi
                wsbase[ww, s] = st.bucket_col0[pp, s] * P + o
                o += int(ecnt[ww, s])

    # x table (dis-scaled, padded rows) in remapped row order
    xtil = np.zeros((NPAD, P), np.float32)
    xtil[remap[:N], :F[0]] = (np.asarray(inp["x"], np.float64)
                              * dis[:, None]).astype(np.float32)
    xt = xtil.astype(bf16)

    cores = []
    for c in range(NCORES):
        m = core == c
        sl, dl, ww, ss = lidx[m], dloc[m], w[m], seg[m]
        # sort by (piece, seg, window) = layout order; src row as tiebreak
        # so each bucket's gather reads ascend through HBM
        okey = ((ww // CW) * NSEG + ss) * NW + ww
        order = np.lexsort((sl, okey))
        sl, dl, ww, ss = (a[order] for a in (sl, dl, ww, ss))
        bid = ww * NSEG + ss
        cnts = np.bincount(bid, minlength=NW * NSEG)
        # j within (w, s) sub-range
        sk = np.argsort(bid, kind="stable")
        inv = np.empty_like(sk)
        inv[sk] = np.arange(len(sk))
        csum = np.concatenate([[0], np.cumsum(cnts)])
        j = inv - csum[bid]
        slot = wsbase[ww, ss] + j
        colv = slot // P
        ioffe = P * ((ww % CW) - st.colw[colv])
        assert ioffe.min() >= 0 and ioffe.max() <= P
        eidx = np.zeros(st.totslot, np.int16)
        edl = np.full(st.totslot, -1.0, np.float32)
        eidx[slot] = sl.astype(np.int16)
        edl[slot] = (dl + ioffe).astype(np.float32)
        # idx wrap: slot i -> [i%16, i//16], replicated over 8 groups of 16
        idx16 = eidx.reshape(-1, 16).T.copy()          # [16, totslot/16]
        idx_full = np.tile(idx16, (8, 1))
        # grid wrap: slot i -> [i%128, i//128]
        edl_g = np.ascontiguousarray(
            edl.reshape(-1, P).T).astype(bf16)
        nid = np.arange(SHARD) + c * SHARD
        gl = np.where(nid < N, batch[np.minimum(nid, N - 1)], -1.0)
        gloc = np.ascontiguousarray(
            gl.reshape(NW, P).T).astype(bf16)          # [128, NW]
        disn = np.where(nid < N, dis[np.minimum(nid, N - 1)], 1.0)
        disv = np.ascontiguousarray(
            disn.reshape(NW, P).T).astype(np.float32)  # [128, NW], dis
        disq = np.ascontiguousarray(
            (disn * disn).reshape(NW, P).T).astype(np.float32)  # 1/deg
        sdeg = (1.0 / disn).astype(bf16).reshape(1, SHARD)
        # self rows: x~ of own shard, in shard order, as [NW*P, P]
        xself = np.zeros((SHARD, P), np.float32)
        v = nid < N
        xself[v] = xtil[remap[nid[v]]]
        xself = xself.astype(bf16)
        cores.append(dict(eidx=idx_full, edl=edl_g, gloc=gloc,
                          disv=disv, disq=disq, sdeg=sdeg, xself=xself))

    cnt_g = np.bincount(batch, minlength=G).astype(np.float32)
    invc = np.zeros((P, 1), np.float32)
    invc[:G, 0] = 1.0 / np.maximum(cnt_g, 1.0)

    iota = np.tile(np.arange(2 * P, dtype=np.float32), (P, 1)).astype(bf16)
    ident = np.eye(P, dtype=np.float32).astype(bf16)

    def a2(x, dt):
        return np.ascontiguousarray(np.asarray(x), dtype=dt)

    wts = dict(
        w1a=np.concatenate([a2(inp["W1"], bf16), a2(inp["b1"], bf16)[None]], 0),
        w2a=np.concatenate([a2(inp["W2"], bf16), a2(inp["b2"], bf16)[None]], 0),
        w3a=np.concatenate([a2(inp["W3"], bf16), a2(inp["b3"], bf16)[None]], 0),
        fw1=a2(inp["fW1"], bf16),
        fb1c=a2(inp["fb1"], np.float32).reshape(-1, 1),
        fw2=a2(inp["fW2"], bf16),
        invc=invc,
        iota=iota,
        ident=ident,
        xt=xt,
    )
    fb2 = float(np.asarray(inp["fb2"]).ravel()[0])
    return st, cores, wts, fb2


# ---------------------------------------------------------------- bass build

def _patch_dma_gather():
    """Relax bass's `elem_size_bytes % 256 == 0` assert on dma_gather for the
    non-transpose path. The 256B constraint is a transpose-mode (xbar)
    restriction; the hardware encoding only requires the row STRIDE to be a
    256B multiple (stride_bytes_256), and the Q7 ucode's non-transpose
    descriptor generation is byte-granular in elem_size. Fetching only
    F_in*2 bytes per edge instead of 256B cuts gather HBM traffic ~3x."""
    import inspect
    import textwrap
    import concourse.bass as cbass
    if getattr(cbass.BassGpSimd.dma_gather, "_small_elem_ok", False):
        return
    src = textwrap.dedent(inspect.getsource(cbass.BassGpSimd.dma_gather))
    old = "elem_size_bytes > 0 and elem_size_bytes % 256 == 0"
    assert old in src, "bass dma_gather source changed; update patch"
    src = src.replace(old, "elem_size_bytes > 0")
    ns = {}
    exec(compile(src, "<dma_gather_small_elem>", "exec"), cbass.__dict__, ns)
    f = ns["dma_gather"]
    f._small_elem_ok = True
    cbass.BassGpSimd.dma_gather = f


def build_bass(st, fb2):
    import concourse.bacc as bacc
    import concourse.bass as bass
    import concourse.mybir as mybir
    import concourse.tile as tile

    _patch_dma_gather()

    dt = mybir.dt
    AF = mybir.ActivationFunctionType
    OP = mybir.AluOpType
    F0, F1, F2, F3 = F
    FMAX = max(F0, F1, F2)
    SBW = st.sbmax + CW - 1                  # S tile: bucket cols + head cols

    nc = bacc.Bacc("TRN2", target_bir_lowering=False, debug=False,
                   enable_asserts=False, num_devices=NCORES,
                   num_swdge_queues=4)

    # ---- I/O
    xt_d = nc.dram_tensor("xt", [NPAD, P], dt.bfloat16, kind="ExternalInput")
    eidx_d = nc.dram_tensor("eidx", [P, st.totslot // 16], dt.int16,
                            kind="ExternalInput")
    edl_d = nc.dram_tensor("edl", [P, st.totcol], dt.bfloat16,
                           kind="ExternalInput")
    gloc_d = nc.dram_tensor("gloc", [P, NW], dt.bfloat16, kind="ExternalInput")
    disv_d = nc.dram_tensor("disv", [P, NW], dt.float32, kind="ExternalInput")
    disq_d = nc.dram_tensor("disq", [P, NW], dt.float32, kind="ExternalInput")
    sdeg_d = (nc.dram_tensor("sdeg", [1, SHARD], dt.bfloat16,
                             kind="ExternalInput") if st.has_bias else None)
    w1a_d = nc.dram_tensor("w1a", [F0 + 1, F1], dt.bfloat16, kind="ExternalInput")
    w2a_d = nc.dram_tensor("w2a", [F1 + 1, F2], dt.bfloat16, kind="ExternalInput")
    w3a_d = nc.dram_tensor("w3a", [F2 + 1, F3], dt.bfloat16, kind="ExternalInput")
    fw1_d = nc.dram_tensor("fw1", [F3, HID], dt.bfloat16, kind="ExternalInput")
    fb1_d = nc.dram_tensor("fb1c", [HID, 1], dt.float32, kind="ExternalInput")
    fw2_d = nc.dram_tensor("fw2", [HID, 1], dt.bfloat16, kind="ExternalInput")
    invc_d = nc.dram_tensor("invc", [P, 1], dt.float32, kind="ExternalInput")
    iota_d = nc.dram_tensor("iota", [P, 2 * P], dt.bfloat16,
                            kind="ExternalInput")
    ident_d = nc.dram_tensor("ident", [P, P], dt.bfloat16, kind="ExternalInput")
    xself_d = nc.dram_tensor("xself", [SHARD, P], dt.bfloat16,
                             kind="ExternalInput")
    out_d = nc.dram_tensor("out", [1, P], dt.float32, kind="ExternalOutput")
    pdbg_d = nc.dram_tensor("pooled_dbg", [P, F3], dt.float32,
                            kind="ExternalOutput")

    rg = [list(range(NCORES))]

    with tile.TileContext(nc) as tc:
        with (
            tc.tile_pool(name="res", bufs=1) as res,
            tc.tile_pool(name="msgs", bufs=3) as msgsp,
            tc.tile_pool(name="sbp", bufs=6) as sbp,
            tc.tile_pool(name="sp", bufs=3) as sp,
            tc.tile_pool(name="selfp", bufs=14) as selfp,
            tc.tile_pool(name="work", bufs=3) as work,
            tc.tile_pool(name="hw", bufs=3) as hwp,
            tc.tile_pool(name="pa_ps", bufs=2, space="PSUM") as pa_ps,
            tc.tile_pool(name="p2_ps", bufs=2, space="PSUM") as p2_ps,
            tc.tile_pool(name="pool_ps", bufs=1, space="PSUM") as pool_ps,
            tc.tile_pool(name="head_ps", bufs=1, space="PSUM") as head_ps,
            tc.tile_pool(name="dram", bufs=1, space="DRAM") as dram,
        ):
            # ---- persistent SBUF state
            eidx = res.tile([P, st.totslot // 16], dt.int16)
            edl = res.tile([P, st.totcol], dt.bfloat16)
            gloc = res.tile([P, NW], dt.bfloat16)
            disv = res.tile([P, NW], dt.float32)
            disq = res.tile([P, NW], dt.float32)
            sdeg = (res.tile([1, SHARD], dt.bfloat16)
                    if st.has_bias else None)
            w1a = res.tile([F0 + 1, F1], dt.bfloat16)
            w2a = res.tile([F1 + 1, F2], dt.bfloat16)
            w3a = res.tile([F2 + 1, F3], dt.bfloat16)
            fw1a = res.tile([F3 // 2, HID], dt.bfloat16)
            fw1b = res.tile([F3 // 2, HID], dt.bfloat16)
            fb1c = res.tile([HID, 1], dt.float32)
            fw2 = res.tile([HID, 1], dt.bfloat16)
            invc = res.tile([P, 1], dt.float32)
            iota_b = res.tile([P, 2 * P], dt.bfloat16)
            ident = res.tile([P, P], dt.bfloat16)
            b1r = res.tile([1, F1], dt.bfloat16)
            b2r = res.tile([1, F2], dt.bfloat16)
            b3r = res.tile([1, F3], dt.bfloat16)
            loads = [(eidx, eidx_d), (edl, edl_d),
                     (gloc, gloc_d), (disv, disv_d), (disq, disq_d),
                     (w1a, w1a_d), (w2a, w2a_d),
                     (w3a, w3a_d), (fb1c, fb1_d), (fw2, fw2_d),
                     (invc, invc_d), (iota_b, iota_d), (ident, ident_d)]
            if st.has_bias:
                loads.append((sdeg, sdeg_d))
            for sb, dr in loads:
                nc.sync.dma_start(out=sb[:], in_=dr[:])
            nc.sync.dma_start(out=b1r[:], in_=w1a_d[F0:F0 + 1, :])
            nc.sync.dma_start(out=b2r[:], in_=w2a_d[F1:F1 + 1, :])
            nc.sync.dma_start(out=b3r[:], in_=w3a_d[F2:F2 + 1, :])
            nc.sync.dma_start(out=fw1a[:], in_=fw1_d[0:F3 // 2, :])
            nc.sync.dma_start(out=fw1b[:], in_=fw1_d[F3 // 2:, :])

            # ---- DRAM tables / buffers (gather tables have 256B rows)
            h1s = dram.tile([NW, P, P], dt.bfloat16)     # shard out, 256B rows
            h2s = dram.tile([NW, P, P], dt.bfloat16)
            h1t = dram.tile([NPAD, P], dt.bfloat16)
            h2t = dram.tile([NPAD, P], dt.bfloat16)
            pool_pt = dram.tile([P, F3], dt.float32)
            pool_rd = dram.tile([P, F3], dt.float32, addr_space="Shared")

            pool_acc = pool_ps.tile([P, F3], dt.float32)

            # cols per dma_gather: with single_packet=False the ucode handles
            # ~8k idxs/inst; >=17920 crashes. 40 cols = 5120 idxs is safe.
            GMAX = 40
            qload = [0, 0, 0, 0]   # greedy idx-count balance across queues

            def gathers(tbl, p, F_in, msgs=None, segs=tuple(range(NSEG))):
                """Issue the per-segment dma_gathers for piece p (chunked to
                <=GMAX*128 idxs per instruction); returns the msgs tile.
                Only F_in columns are fetched per row (small-elem gather)."""
                if msgs is None:
                    msgs = msgsp.tile([P, st.cmax, F_in], dt.bfloat16,
                                      tag=f"m{F_in}", name="msgs",
                                      bufs=4 if F_in <= 40 else 3)
                s0 = st.piece_col0[p] * P
                for s in segs:
                    loff, ncols = st.gath[p][s]
                    for c0 in range(0, ncols, GMAX):
                        nc_ = min(GMAX, ncols - c0)
                        ni = nc_ * P
                        slot0 = s0 + (loff + c0) * P
                        q = min(range(4), key=lambda i: qload[i])
                        qload[q] += ni
                        nc.gpsimd.dma_gather(
                            out_ap=msgs[:, loff + c0:loff + c0 + nc_, :],
                            in_ap=tbl[REG_BASE[s]:REG_BASE[s] + REG_SIZE[s],
                                      :F_in],
                            idxs_ap=eidx[:, slot0 // 16:(slot0 + ni) // 16],
                            num_idxs=ni, num_idxs_reg=ni, elem_size=F_in,
                            elem_step=P,
                            single_packet=False,
                            queue_num=q)
                return msgs

            def compute_piece(msgs, F_in, F_out, waug, brow, shard_out, p,
                              last, selfrow):
                hs = []
                for wi in range(CW):
                    w = p * CW + wi
                    hp = selfp.tile([P, P], dt.bfloat16, tag="hp", name="hp")
                    nc.sync.dma_start(out=hp[:], in_=selfrow(w))
                    hs.append(hp)
                # one merged S build per (piece, seg) bucket + per-head cols
                Sb = [None] * NSEG
                for s in range(NSEG):
                    loff, bcols = st.gath[p][s]
                    if bcols == 0:
                        continue
                    sheads = st.heads[p][s]
                    Sb[s] = sbp.tile([P, SBW, P], dt.bfloat16, tag="Sb",
                                     name="Sb")
                    gc0 = int(st.bucket_col0[p, s])
                    nc.vector.tensor_tensor(
                        out=Sb[s][:, :bcols, :],
                        in0=edl[:, gc0:gc0 + bcols, None].broadcast_to(
                            [P, bcols, P]),
                        in1=iota_b[:, None, :P].broadcast_to([P, bcols, P]),
                        op=OP.is_equal)
                    for hi, lc in enumerate(sheads):
                        nc.vector.tensor_tensor(
                            out=Sb[s][:, bcols + hi:bcols + hi + 1, :],
                            in0=edl[:, gc0 + lc:gc0 + lc + 1, None]
                            .broadcast_to([P, 1, P]),
                            in1=iota_b[:, None, P:2 * P].broadcast_to(
                                [P, 1, P]),
                            op=OP.is_equal)
                for wi in range(CW):
                    w = p * CW + wi
                    runs = st.wruns[p * CW + wi]
                    tw = sum(rn[1] for rn in runs)
                    pa = pa_ps.tile([FMAX, P], dt.float32, tag="pa",
                                    name="pa")[:F_in]
                    # self-loop contribution: pa[:, d] += hprev[d, :F_in]
                    nc.tensor.matmul(out=pa[:], lhsT=hs[wi][:, :F_in],
                                     rhs=ident[:], start=True, stop=(tw == 0))
                    k = 0
                    for (mloff, rn, s, scol) in runs:
                        for j in range(rn):
                            nc.tensor.matmul(
                                out=pa[:], lhsT=msgs[:, mloff + j, :F_in],
                                rhs=Sb[s][:, scol + j, :], start=False,
                                stop=(k == tw - 1))
                            k += 1
                    aggT = work.tile([FMAX, P], dt.bfloat16, tag="aggT",
                                     name="aggT")[:F_in]
                    nc.scalar.copy(out=aggT[:], in_=pa[:])
                    p2 = p2_ps.tile([P, F3], dt.float32, tag="p2",
                                    name="p2")[:, :F_out]
                    nc.tensor.matmul(out=p2[:], lhsT=aggT[:], rhs=waug[:F_in, :],
                                     start=True, stop=not st.has_bias)
                    if st.has_bias:
                        nc.tensor.matmul(out=p2[:],
                                         lhsT=sdeg[:, w * P:(w + 1) * P],
                                         rhs=brow[:], start=False, stop=True)
                    if not last:
                        # store dis*relu(agg@W+b) = relu(dis^2 * p2)
                        h = hwp.tile([P, P], dt.bfloat16, tag="h", name="h")
                        nc.scalar.activation(h[:, :F_out], p2[:], AF.Relu,
                                             scale=disq[:, w:w + 1])
                        nc.sync.dma_start(out=shard_out[w, :, :], in_=h[:])
                    else:
                        h = hwp.tile([P, F3], dt.bfloat16, tag="h", name="h")
                        nc.scalar.activation(h[:], p2[:], AF.Relu,
                                             scale=disv[:, w:w + 1])
                        Sg = sp.tile([P, P], dt.bfloat16, tag="Sg", name="Sg")
                        nc.vector.tensor_tensor(
                            out=Sg[:],
                            in0=gloc[:, w:w + 1].broadcast_to([P, P]),
                            in1=iota_b[:, :P], op=OP.is_equal)
                        nc.tensor.matmul(out=pool_acc[:], lhsT=Sg[:], rhs=h[:],
                                         start=(w == 0), stop=(w == NW - 1))

            def allgather_seg(shard, ptab, s):
                a, b = REG_BASE[s], REG_BASE[s] + REG_SIZE[s]
                nc.gpsimd.collective_compute(
                    "AllGather", mybir.AluOpType.bypass, replica_groups=rg,
                    ins=[shard[SEG_WSTART[s]:SEG_WSTART[s] + SEG_W[s], :, :].opt()],
                    outs=[ptab[a:b, :].opt()])

            PRE_N = 2   # pieces of the next layer pregathered (segs 0-2)

            def layer(tbl, F_in, F_out, waug, brow, shard_out, ag,
                      selfrow, pre=None, nxt_tbl=None, nxt_F=0):
                # ag(s): AllGather of THIS layer's output segment s; issued on
                # the Pool queue two pieces after segment s's last piece so
                # the trigger's wait on compute h-writes doesn't stall the
                # gather stream. Segment 3's fires after the loop.
                # pre: {piece: msgs} pregathered (segs 0-2) by the previous
                # layer; their seg-3 chunks are issued here after ag(3) of the
                # previous layer has been queued. nxt_tbl: gather table of the
                # NEXT layer; pieces 0..PRE_N-1 segs 0-2 are issued right
                # after this layer's last own gathers to keep the SWDGE
                # queues busy across the layer boundary.
                last = shard_out is None
                pre = pre or {}
                nxt_pre = {}
                for p in range(NP):
                    if p in pre:
                        msgs = gathers(tbl, p, F_in, msgs=pre[p], segs=(3,))
                    else:
                        msgs = gathers(tbl, p, F_in)
                    if ag is not None:
                        for s in range(3):
                            if p == 4 * s + 5:
                                ag(s)
                    if p == NP - 1 and nxt_tbl is not None:
                        for p2 in range(PRE_N):
                            nxt_pre[p2] = gathers(nxt_tbl, p2, nxt_F,
                                                  segs=(0, 1, 2))
                    compute_piece(msgs, F_in, F_out, waug, brow, shard_out,
                                  p, last, selfrow)
                if ag is not None:
                    ag(3)
                return nxt_pre

            dbg_layers = int(os.environ.get("GCN_DEBUG_LAYERS", "3"))
            n_repeat = int(os.environ.get("GCN_REPEAT", "1"))
            for _rep in range(n_repeat):
                pre = None
                if dbg_layers >= 1:
                    pre = layer(xt_d, F0, F1, w1a, b1r, h1s,
                                (lambda s: allgather_seg(h1s, h1t, s))
                                if dbg_layers >= 2 else None,
                                lambda w: xself_d[w * P:(w + 1) * P, :],
                                nxt_tbl=h1t if dbg_layers >= 2 else None,
                                nxt_F=F1)
                if dbg_layers >= 2:
                    pre = layer(h1t, F1, F2, w2a, b2r, h2s,
                                (lambda s: allgather_seg(h2s, h2t, s))
                                if dbg_layers >= 3 else None,
                                lambda w: h1s[w, :, :],
                                pre=pre,
                                nxt_tbl=h2t if dbg_layers >= 3 else None,
                                nxt_F=F2)
                if dbg_layers >= 3:
                    layer(h2t, F2, F3, w3a, b3r, None, None,
                          lambda w: h2s[w, :, :], pre=pre)
                if dbg_layers < 3:
                    # stub tail: emit outputs without pool/head
                    dsrc = h1s if dbg_layers >= 1 else None
                    pz = work.tile([P, F3], dt.float32, tag="poolr")
                    if dsrc is not None:
                        nc.vector.memset(pz[:], 0.0)
                        hdbg = work.tile([P, F1], dt.bfloat16, tag="hdbg")
                        nc.sync.dma_start(out=hdbg[:], in_=dsrc[0, :, :F1])
                        nc.vector.tensor_copy(out=pz[:, :F1], in_=hdbg[:])
                    else:
                        nc.vector.memset(pz[:], 0.0)
                    nc.sync.dma_start(out=pdbg_d[:], in_=pz[:])
                    o1 = work.tile([1, P], dt.float32, tag="outs")
                    nc.vector.memset(o1[:], 1.0)
                    nc.sync.dma_start(out=out_d[:], in_=o1[:])

                if dbg_layers >= 3:
                    # ---- pooling partial -> AllReduce -> mean
                    psb = work.tile([P, F3], dt.float32, tag="psb")
                    nc.scalar.copy(out=psb[:], in_=pool_acc[:])
                    nc.sync.dma_start(out=pool_pt[:], in_=psb[:])
                    nc.gpsimd.collective_compute(
                        "AllReduce", mybir.AluOpType.add, replica_groups=rg,
                        ins=[pool_pt.opt()], outs=[pool_rd.opt()])
                    poolr = work.tile([P, F3], dt.float32, tag="poolr")
                    nc.sync.dma_start(out=poolr[:], in_=pool_rd[:])
                    nc.sync.dma_start(out=pdbg_d[:], in_=poolr[:])
                    pooled = work.tile([P, F3], dt.bfloat16, tag="pooled")
                    nc.scalar.activation(pooled[:], poolr[:], AF.Copy, scale=invc[:])

                    # ---- head: z1 = relu(pooled @ fW1 + fb1); z2 = z1 @ fW2 + fb2
                    ptA_ps = head_ps.tile([F3 // 2, P], dt.bfloat16, tag="pt")
                    nc.tensor.transpose(out=ptA_ps[:], in_=pooled[:, :F3 // 2],
                                        identity=ident[:])
                    ptA = work.tile([F3 // 2, P], dt.bfloat16, tag="ptA")
                    nc.scalar.copy(out=ptA[:], in_=ptA_ps[:])
                    ptB_ps = head_ps.tile([F3 // 2, P], dt.bfloat16, tag="pt")
                    nc.tensor.transpose(out=ptB_ps[:], in_=pooled[:, F3 // 2:],
                                        identity=ident[:])
                    ptB = work.tile([F3 // 2, P], dt.bfloat16, tag="ptB")
                    nc.scalar.copy(out=ptB[:], in_=ptB_ps[:])

                    z1_ps = head_ps.tile([HID, P], dt.float32, tag="z1")
                    nc.tensor.matmul(out=z1_ps[:], lhsT=fw1a[:], rhs=ptA[:],
                                     start=True, stop=False)
                    nc.tensor.matmul(out=z1_ps[:], lhsT=fw1b[:], rhs=ptB[:],
                                     start=False, stop=True)
                    z1 = work.tile([HID, P], dt.bfloat16, tag="z1s")
                    nc.scalar.activation(z1[:], z1_ps[:], AF.Relu, bias=fb1c[:])

                    z2_ps = head_ps.tile([1, P], dt.float32, tag="z2")
                    nc.tensor.matmul(out=z2_ps[:], lhsT=fw2[:], rhs=z1[:],
                                     start=True, stop=True)
                    z2 = work.tile([1, P], dt.float32, tag="z2s")
                    nc.scalar.activation(z2[:], z2_ps[:], AF.Copy, bias=float(fb2))
                    # softmax over a width-1 axis == 1.0 for finite logits
                    outs = work.tile([1, P], dt.float32, tag="outs")
                    nc.vector.tensor_tensor(out=outs[:], in0=z2[:], in1=z2[:],
                                            op=OP.is_equal)
                    nc.sync.dma_start(out=out_d[:], in_=outs[:])

    nc.compile()
    return nc


# ---------------------------------------------------------------- run

_CACHE = {}


def _get_nc(st, fb2):
    import os as _os
    key = (st.key(), fb2, _os.environ.get('GCN_DEBUG_LAYERS', '3'))
    if key not in _CACHE:
        _CACHE[key] = build_bass(st, fb2)
    return _CACHE[key]


def make_in_maps(inputs):
    st, cores, wts, fb2 = build_host_data(inputs)
    in_maps = [dict(**cores[c], **wts) for c in range(NCORES)]
    return st, in_maps, fb2


LAST_RESULTS = None


def kernel(**inputs):
    global LAST_RESULTS
    st, in_maps, fb2 = make_in_maps(inputs)
    nc = _get_nc(st, fb2)
    from concourse.bass_utils import run_bass_kernel_spmd
    res = run_bass_kernel_spmd(nc, in_maps, core_ids=list(range(NCORES)))
    LAST_RESULTS = res
    out = np.asarray(res.results[0]["out"]).reshape(P)[:G]
    return out.reshape(G, 1).astype(np.float32)
